# revision 30
# baseline (speedup 1.0000x reference)
"""Trainium2 Bass kernel for nn_Loss_factory_12429635355015.

Loss = NLLSurv + CohortLoss(intra + inter) over a [4, 8192, 4, 256] cohort bank.

Strategy (memory-bound, 8 NeuronCores):
  - Shard cohort_bank along the N (bank-entry) axis: each core streams its
    16 MiB shard once at HBM line rate via SWDGE cast-DMAs (f32 -> bf16).
  - Per tile (n entries, 4 per partition): DVE does the component-sum and
    sum-of-squares, ACT does rsqrt + exp(+accum), PE transposes S and runs
    the anchor matmul.  Per-tile engine busy is kept below the ~5.75us/tile
    DMA floor so the HBM stream is the only bottleneck.
  - The scalar NLL + intra terms are emitted AFTER the main loop so their
    long serial dependency chains get LOW scheduler priority and fill idle
    engine slots instead of gating the DVE progress counters that recycle
    DMA buffers (deep tile pools absorb the remaining jitter).
  - Each core outputs [ep_partial, en_partial, nll+intra]; the host sums the
    two scalars across cores (the 'all-reduce two scalars' step) and applies
    the final -log((ep+eps)/(ep+en+eps)).
"""

import math
import os
import sys

import numpy as np

for _p in ("/opt/trn_rl_repo",):
    if _p not in sys.path and os.path.isdir(_p):
        sys.path.insert(0, _p)

import concourse.bacc as bacc
import concourse.tile as tile
from concourse import mybir
from concourse.bass_utils import run_bass_kernel_spmd

# Pin every activation to the one table set that contains all functions this
# kernel uses (Square/Ln/Exp/Copy/Abs/Identity). Without this, Bacc's
# first-match set selection alternates between sets (Ln lives outside the
# default exp set) and reloads the ACT tables ~1.3us per switch every tile.
_ACT_SET = "natural_log_exp_and_others"


def _pin_act_tables():
    import functools
    import concourse.hw_specs as hw_specs
    if getattr(hw_specs.get_activation_tables, "_pinned", False):
        return
    orig = hw_specs.get_activation_tables

    @functools.cache
    def pinned(arch):
        tabs = orig(arch)
        return {k: (v if k == _ACT_SET else set()) for k, v in tabs.items()}

    pinned._pinned = True
    hw_specs.get_activation_tables = pinned
    bacc.get_activation_tables = pinned


_pin_act_tables()

F32 = mybir.dt.float32
AF = mybir.ActivationFunctionType

# Problem constants (hardcoded per spec).
B = 64            # batch
K = 4             # n_cls
C = 256           # feature dim
NB = 8192         # bank entries per class (global)
NCORES = 8
NSH = NB // NCORES          # 1024 bank entries per class per core
ROWS = K * NSH              # 4096 rows of [4*256] per core
EPS_NLL = 1e-7
EPS_COH = 1e-8


def _build():
    nc = bacc.Bacc("TRN2", target_bir_lowering=False, debug=False,
                   enable_asserts=False, num_devices=NCORES)

    bank = nc.dram_tensor("bank", [ROWS, 1024], F32, kind="ExternalInput")
    indiv = nc.dram_tensor("indiv", [B, 1024], F32, kind="ExternalInput")
    gp = nc.dram_tensor("gp", [B, 512], F32, kind="ExternalInput")
    haz = nc.dram_tensor("haz", [B, K], F32, kind="ExternalInput")
    spad = nc.dram_tensor("spad", [B, K + 1], F32, kind="ExternalInput")
    ohy = nc.dram_tensor("ohy", [B, K + 1], F32, kind="ExternalInput")
    ohy1 = nc.dram_tensor("ohy1", [B, K + 1], F32, kind="ExternalInput")
    oh4 = nc.dram_tensor("oh4", [B, K], F32, kind="ExternalInput")
    cfs = nc.dram_tensor("cfs", [B, 2], F32, kind="ExternalInput")

    out_d = nc.dram_tensor("out_f", [B, 4], F32, kind="ExternalOutput")

    import ml_dtypes
    ident_bf_d = nc.inline_tensor(np.eye(128, dtype=ml_dtypes.bfloat16), "ident_bf")

    v = nc.vector
    a = nc.scalar

    with tile.TileContext(nc) as tc:
        from contextlib import ExitStack
        with ExitStack() as ctx:
            const = ctx.enter_context(tc.tile_pool(name="const", bufs=1))
            small = ctx.enter_context(tc.tile_pool(name="small", bufs=1))
            tpool = ctx.enter_context(tc.tile_pool(name="T", bufs=8))
            spool = ctx.enter_context(tc.tile_pool(name="S", bufs=6))
            stpool = ctx.enter_context(tc.tile_pool(name="STsb", bufs=6))
            epool = ctx.enter_context(tc.tile_pool(name="esb", bufs=6))
            sqpool = ctx.enter_context(tc.tile_pool(name="sq", bufs=6))
            ps_st = ctx.enter_context(tc.tile_pool(name="ps_st", bufs=2, space="PSUM"))
            ps_p = ctx.enter_context(tc.tile_pool(name="ps_p", bufs=3, space="PSUM"))
            ps_one = ctx.enter_context(tc.tile_pool(name="ps_one", bufs=1, space="PSUM"))

            BF16 = mybir.dt.bfloat16
            # Small inputs go through the SAME SWDGE ring as the tile stream:
            # HWDGE transfers starve behind a saturated SWDGE stream (the 16
            # SDMA engines round-robin, and the stream has a deep backlog), so
            # the two loop-critical inputs are queued BEFORE the stream and
            # the scalar-term inputs are interleaved between tile issues.
            ident_bf = const.tile([128, 128], BF16)
            nc.gpsimd.dma_start(out=ident_bf[:], in_=ident_bf_d[:])
            # indiv/gp are cast to bf16 during DMA: halves the prologue's
            # big elementwise ops on DVE (anchor/intra numerics tolerate it)
            ind_sb = small.tile([B, 1024], BF16)
            nc.gpsimd.dma_start(out=ind_sb[:], in_=indiv[:])

            sm_shapes = {"haz": K, "spad": K + 1, "ohy": K + 1, "ohy1": K + 1,
                         "oh4": K, "cfs": 2}
            sm_tiles = {n: small.tile([B, w], F32, name=f"sm_{n}")
                        for n, w in sm_shapes.items()}
            sm_tiles["gp"] = small.tile([B, 512], BF16, name="sm_gp")
            sm_src = {"haz": haz, "spad": spad, "ohy": ohy, "ohy1": ohy1,
                      "oh4": oh4, "cfs": cfs, "gp": gp}
            sm_sched = [["haz", "spad", "gp"], ["ohy", "ohy1"],
                        ["oh4", "cfs"], [], [], [], [], []]

            # ---------- hoisted DMA-issue block ----------
            # All DMA issues are emitted first in program order: with
            # bufs=8 none of them carries a buffer-recycle wait, so the
            # static schedule keeps the GPSIMD queue pure-DMA and the
            # stream self-paces at HBM line rate no matter what the
            # compute engines are doing.
            T_tiles = []
            for t in range(8):
                T_sb = tpool.tile([128, 4096], BF16, name=f"T{t}", tag="T")
                src = bank[t * 512:(t + 1) * 512, :].rearrange(
                    "(p e) x -> p e x", e=4)
                # SWDGE cast-DMA: f32 HBM -> bf16 SBUF at line rate
                nc.gpsimd.dma_start(
                    out=T_sb.rearrange("p (e x) -> p e x", e=4), in_=src)
                for name in sm_sched[t]:
                    nc.gpsimd.dma_start(out=sm_tiles[name][:],
                                        in_=sm_src[name][:])
                T_tiles.append(T_sb)

            # ---------- anchors: A = l2norm(mean_j indiv[b,j,:]) ----------
            # (must precede the loop: at_sb feeds every tile's matmul)
            iv = ind_sb.rearrange("p (j c) -> p j c", j=4)
            asum = small.tile([B, C], BF16)
            atmp = small.tile([B, C], BF16)
            v.tensor_add(asum[:], iv[:, 0, :], iv[:, 1, :])
            v.tensor_add(atmp[:], iv[:, 2, :], iv[:, 3, :])
            v.tensor_add(asum[:], asum[:], atmp[:])
            sqa = small.tile([B, C], F32)
            ssa = small.tile([B, 1], F32)
            a.activation(sqa[:], asum[:], AF.Square, accum_out=ssa[:])
            lna = small.tile([B, 1], F32)
            a.activation(lna[:], ssa[:], AF.Ln)
            rsa = small.tile([B, 1], F32)
            a.activation(rsa[:], lna[:], AF.Exp, scale=-0.5)
            v.tensor_scalar_mul(asum[:], asum[:], rsa[:])
            at_ps = ps_one.tile([128, 2, B], BF16, tag="at")
            for h in range(2):
                nc.tensor.transpose(at_ps[:, h, :],
                                    asum[:, h * 128:(h + 1) * 128],
                                    ident_bf[0:B, 0:B])
            at_sb = const.tile([128, 2, B], BF16)
            a.copy(at_sb[:], at_ps[:])

            # ---------- main loop over bank tiles ----------
            # 8 DMA transfers of 512 entries each (fewest ramps); the LAST
            # transfer's compute is chunked 256/128/128 along the e axis so
            # the serial drain chains after the HBM stream ends are short.
            # (Entries within a DMA tile all belong to one class, and the
            # e-axis chunking just partitions the entry set.)
            E_sb = small.tile([B, K], F32)
            F = small.tile([B, 4], F32)
            v.memset(E_sb[:], 0.0)
            v.memset(F[:], 0.0)
            dma_chunks = [[(0, 4)]] * 7 + [[(0, 2), (2, 1), (3, 1)]]
            for t in range(8):
                k = t // 2
                Tv = T_tiles[t].rearrange("p (e j c) -> p e j c", e=4, j=4)
                # Pin tile t's compute to its REAL data-arrival time: the
                # scheduler's DMA cost model is ~20% optimistic, and the
                # resulting too-eager static order makes the in-order
                # engines lag the stream by ~10us at the drain.
                tc.tile_set_cur_wait(0.0118 + 0.006 * (t + 1))
                for e0, ne in dma_chunks[t]:
                    nt = ne * 128
                    Tc = Tv[:, e0:e0 + ne, :, :]
                    S_sb = spool.tile([128, 1024], BF16)
                    Sv = S_sb[:, :ne * C].rearrange("p (e c) -> p e c", e=ne)
                    tmp = spool.tile([128, 1024], BF16, tag="tmp")
                    tv = tmp[:, :ne * C].rearrange("p (e c) -> p e c", e=ne)
                    v.tensor_add(Sv[:], Tc[:, :, 0, :], Tc[:, :, 1, :])
                    v.tensor_add(tv[:], Tc[:, :, 2, :], Tc[:, :, 3, :])
                    v.tensor_add(Sv[:], Sv[:], tv[:])
                    # sum-of-squares on DVE (Q7 compute interferes with the
                    # SWDGE stream, so GPSIMD stays pure-DMA)
                    sq = sqpool.tile([128, 1024], BF16)
                    sqv = sq[:, :ne * C].rearrange("p (e c) -> p e c", e=ne)
                    v.tensor_mul(sqv[:], Sv[:], Sv[:])
                    ssum4 = spool.tile([128, 4], F32, tag="ssum")
                    v.reduce_sum(ssum4[:, :ne], sqv[:], axis=mybir.AxisListType.X)
                    rh4 = spool.tile([128, 4], F32, tag="rh4")
                    a.activation(rh4[:, :ne], ssum4[:, :ne], AF.Ln)
                    a.activation(rh4[:, :ne], rh4[:, :ne], AF.Exp, scale=-0.5)
                    # normalize S rows in place (per-entry 1/||S||); split the
                    # four per-e scalings across DVE and ACT to balance load
                    for e in range(ne):
                        if e % 2 == 0:
                            v.tensor_scalar_mul(Sv[:, e, :], Sv[:, e, :],
                                                rh4[:, e:e + 1])
                        else:
                            a.activation(Sv[:, e, :], Sv[:, e, :], AF.Copy,
                                         scale=rh4[:, e:e + 1])
                    # transpose e-groups into [c, n] chunks (h = c-half)
                    st_ps = [ps_st.tile([128, 512], BF16, name=f"stps{h}",
                                        tag=f"stps{h}") for h in range(2)]
                    for e in range(ne):
                        for h in range(2):
                            nc.tensor.transpose(
                                st_ps[h][:, e * 128:(e + 1) * 128],
                                S_sb[:, e * C + h * 128: e * C + (h + 1) * 128],
                                ident_bf[:])
                    p_ps = ps_p.tile([B, 512], F32)
                    for h in range(2):
                        st_sb = stpool.tile([128, 512], BF16)
                        a.copy(st_sb[:, :nt], st_ps[h][:, :nt])
                        nc.tensor.matmul(p_ps[:, :nt], at_sb[:, h, :],
                                         st_sb[:, :nt],
                                         start=(h == 0), stop=(h == 1))
                    e_sb = epool.tile([B, 512], F32)
                    et = epool.tile([B, 1], F32, tag="et")
                    # exp(sims/tau) with the row-sum accumulated in the same op
                    a.activation(e_sb[:, :nt], p_ps[:, :nt], AF.Exp, scale=0.5,
                                 accum_out=et[:])
                    v.tensor_add(E_sb[:, k:k + 1], E_sb[:, k:k + 1], et[:])
                if t == 4:
                    contrib = _emit_scalar_terms(nc, tc, small, ind_sb,
                                                 sm_tiles)
            tc.cur_wait_ts = None

            # ---------- epilogue: per-b partial columns, host finishes ----
            oh4_sb = sm_tiles["oh4"]
            t4b = small.tile([B, K], F32)
            v.tensor_mul(t4b[:], E_sb[:], oh4_sb[:])
            v.reduce_sum(F[:, 0:1], t4b[:], axis=mybir.AxisListType.X)
            v.reduce_sum(F[:, 1:2], E_sb[:], axis=mybir.AxisListType.X)
            v.tensor_copy(F[:, 2:3], contrib[:])
            nc.sync.dma_start(out=out_d[:], in_=F[:])

    nc.compile()
    return nc


def _emit_scalar_terms(nc, tc, small, ind_sb, sm):
    """NLL + intra terms (tiny [B,*] math).  Emitted mid-loop with sim-time
    pins at each chain's real input-arrival time, so the scheduler slots the
    long serial chains into true engine-idle windows of the HBM stream."""
    v = nc.vector
    a = nc.scalar
    contrib = _emit_intra(nc, tc, small, ind_sb, sm)

    # ---------- NLL (per-b, b on partitions) ----------
    tc.tile_set_cur_wait(0.027)
    haz_sb = sm["haz"]
    spad_sb = sm["spad"]
    ohy_sb = sm["ohy"]
    ohy1_sb = sm["ohy1"]
    cfs_sb = sm["cfs"]

    t5 = small.tile([B, K + 1], F32)
    t4 = small.tile([B, K], F32)
    sy = small.tile([B, 1], F32)
    hy = small.tile([B, 1], F32)
    sy1 = small.tile([B, 1], F32)
    v.tensor_mul(t5[:], spad_sb[:], ohy_sb[:])
    v.reduce_sum(sy[:], t5[:], axis=mybir.AxisListType.X)
    v.tensor_mul(t4[:], haz_sb[:], ohy_sb[:, 0:K])
    v.reduce_sum(hy[:], t4[:], axis=mybir.AxisListType.X)
    v.tensor_mul(t5[:], spad_sb[:], ohy1_sb[:])
    v.reduce_sum(sy1[:], t5[:], axis=mybir.AxisListType.X)
    for x in (sy, hy, sy1):
        v.tensor_scalar_max(x[:], x[:], EPS_NLL)
    lsy = small.tile([B, 1], F32)
    lhy = small.tile([B, 1], F32)
    lsy1 = small.tile([B, 1], F32)
    a.activation(lsy[:], sy[:], AF.Ln)
    a.activation(lhy[:], hy[:], AF.Ln)
    a.activation(lsy1[:], sy1[:], AF.Ln)
    tu = small.tile([B, 1], F32)
    tcen = small.tile([B, 1], F32)
    negl = small.tile([B, 1], F32)
    v.tensor_add(tu[:], lsy[:], lhy[:])
    v.tensor_mul(tu[:], tu[:], cfs_sb[:, 1:2])      # *(1-cf)
    v.tensor_mul(tcen[:], lsy1[:], cfs_sb[:, 0:1])  # *cf
    v.tensor_add(negl[:], tu[:], tcen[:])           # = -neg_l per b

    # combine: contrib_b = -negl/B + isum/(8B) + 1/B
    isum = contrib.pop("isum")
    c1 = small.tile([B, 1], F32)
    c2 = small.tile([B, 1], F32)
    cb = small.tile([B, 1], F32)
    v.tensor_scalar_mul(c1[:], negl[:], -1.0 / B)
    v.tensor_scalar_mul(c2[:], isum[:], 1.0 / (8 * B))
    v.tensor_add(cb[:], c1[:], c2[:])
    v.tensor_scalar_add(cb[:], cb[:], 1.0 / B)
    return cb


def _emit_intra(nc, tc, small, ind_sb, sm):
    v = nc.vector
    a = nc.scalar
    # ---------- intra cohort term (bf16 inputs) ----------
    tc.tile_set_cur_wait(0.019)
    BF16 = mybir.dt.bfloat16
    gp_sb = sm["gp"]
    sqi = small.tile([B, 1024], BF16)
    v.tensor_mul(sqi[:], ind_sb[:], ind_sb[:])
    ssqi = small.tile([B, 4], F32)
    v.reduce_sum(ssqi[:], sqi.rearrange("p (j c) -> p j c", j=4),
                 axis=mybir.AxisListType.X)
    rsi = small.tile([B, 4], F32)
    a.activation(rsi[:], ssqi[:], AF.Ln)
    a.activation(rsi[:], rsi[:], AF.Exp, scale=-0.5)
    sqg = small.tile([B, 512], BF16)
    v.tensor_mul(sqg[:], gp_sb[:], gp_sb[:])
    ssqg = small.tile([B, 2], F32)
    v.reduce_sum(ssqg[:], sqg.rearrange("p (t c) -> p t c", t=2),
                 axis=mybir.AxisListType.X)
    rsg = small.tile([B, 2], F32)
    a.activation(rsg[:], ssqg[:], AF.Ln)
    a.activation(rsg[:], rsg[:], AF.Exp, scale=-0.5)
    # normalize rows in place (anchor sums already consumed ind_sb)
    for p in range(4):
        v.tensor_scalar_mul(ind_sb[:, p * C:(p + 1) * C],
                            ind_sb[:, p * C:(p + 1) * C], rsi[:, p:p + 1])
    for t in range(2):
        v.tensor_scalar_mul(gp_sb[:, t * C:(t + 1) * C],
                            gp_sb[:, t * C:(t + 1) * C], rsg[:, t:t + 1])
    D = small.tile([B, 8], F32)
    prod = small.tile([B, C], BF16)
    for p in range(4):
        for t in range(2):
            col = p * 2 + t
            v.tensor_mul(prod[:], ind_sb[:, p * C:(p + 1) * C],
                         gp_sb[:, t * C:(t + 1) * C])
            v.reduce_sum(D[:, col:col + 1], prod[:],
                         axis=mybir.AxisListType.X)
    U = small.tile([B, 8], F32)
    a.activation(U[:], D[:], AF.Abs)
    # mask==1 entries (cols 0,1,4,7) use -sim instead of |sim|
    v.tensor_scalar_mul(U[:, 0:2], D[:, 0:2], -1.0)
    v.tensor_scalar_mul(U[:, 4:5], D[:, 4:5], -1.0)
    v.tensor_scalar_mul(U[:, 7:8], D[:, 7:8], -1.0)
    isum = small.tile([B, 1], F32)
    v.reduce_sum(isum[:], U[:], axis=mybir.AxisListType.X)
    return {"isum": isum}


_NC = None


def _get_nc():
    global _NC
    if _NC is None:
        _NC = _build()
    return _NC


def _make_in_maps(hazards, S, indiv, gene, path, cohort_bank, label, c):
    hazards = np.asarray(hazards, dtype=np.float32)
    S = np.asarray(S, dtype=np.float32)
    indiv = np.asarray(indiv, dtype=np.float32)
    gene = np.asarray(gene, dtype=np.float32)
    path = np.asarray(path, dtype=np.float32)
    cohort_bank = np.asarray(cohort_bank, dtype=np.float32)
    label = np.asarray(label)
    c = np.asarray(c)

    oh5 = np.zeros((B, K + 1), np.float32)
    oh5[np.arange(B), label] = 1.0
    oh5b = np.zeros((B, K + 1), np.float32)
    oh5b[np.arange(B), label + 1] = 1.0
    oh4 = oh5[:, :K].copy()
    spad = np.concatenate([np.ones((B, 1), np.float32), S], axis=1)
    cfs = np.stack([c.astype(np.float32), 1.0 - c.astype(np.float32)], axis=1)
    common = dict(
        indiv=np.ascontiguousarray(indiv.reshape(B, -1)),
        gp=np.ascontiguousarray(
            np.concatenate([gene.reshape(B, -1), path.reshape(B, -1)], axis=1)),
        haz=np.ascontiguousarray(hazards),
        spad=np.ascontiguousarray(spad),
        ohy=oh5, ohy1=oh5b, oh4=oh4, cfs=np.ascontiguousarray(cfs),
    )
    bankf = cohort_bank.reshape(K, NB, 1024)
    in_maps = []
    for i in range(NCORES):
        shard = np.ascontiguousarray(
            bankf[:, i * NSH:(i + 1) * NSH, :]).reshape(ROWS, 1024)
        in_maps.append({**common, "bank": shard})
    return in_maps


_LAST_RESULTS = None  # stashed for test.py introspection


def kernel(hazards, S, indiv, gene, path, cohort_bank, label, c):
    global _LAST_RESULTS
    nc = _get_nc()
    in_maps = _make_in_maps(hazards, S, indiv, gene, path, cohort_bank, label, c)
    trace = bool(int(os.environ.get("TRNK_TRACE", "0")))
    res = run_bass_kernel_spmd(nc, in_maps, core_ids=list(range(NCORES)),
                               trace=trace)
    _LAST_RESULTS = res
    outs = np.stack([r["out_f"] for r in res.results])  # [8, B, 4]
    ep_raw = float(outs[:, :, 0].sum())
    rsum_raw = float(outs[:, :, 1].sum())
    ep = ep_raw / (B * NB)
    en = (rsum_raw - ep_raw) / (B * (K - 1) * NB)
    other = float(outs[:, :, 2].sum(axis=1).mean())
    loss = other - math.log((ep + EPS_COH) / (ep + en + EPS_COH))
    return np.float32(loss)


# revision 36
# speedup vs baseline: 1.0309x; 1.0309x over previous
"""Trainium2 Bass kernel for nn_Loss_factory_12429635355015.

Loss = NLLSurv + CohortLoss(intra + inter) over a [4, 8192, 4, 256] cohort bank.

Strategy (memory-bound, 8 NeuronCores):
  - Shard cohort_bank along the N (bank-entry) axis: each core streams its
    16 MiB shard once at HBM line rate via SWDGE cast-DMAs (f32 -> bf16).
  - Per tile (n entries, 4 per partition): DVE does the component-sum and
    sum-of-squares, ACT does rsqrt + exp(+accum), PE transposes S and runs
    the anchor matmul.  Per-tile engine busy is kept below the ~5.75us/tile
    DMA floor so the HBM stream is the only bottleneck.
  - The scalar NLL + intra terms are emitted AFTER the main loop so their
    long serial dependency chains get LOW scheduler priority and fill idle
    engine slots instead of gating the DVE progress counters that recycle
    DMA buffers (deep tile pools absorb the remaining jitter).
  - Each core outputs [ep_partial, en_partial, nll+intra]; the host sums the
    two scalars across cores (the 'all-reduce two scalars' step) and applies
    the final -log((ep+eps)/(ep+en+eps)).
"""

import math
import os
import sys

import numpy as np

for _p in ("/opt/trn_rl_repo",):
    if _p not in sys.path and os.path.isdir(_p):
        sys.path.insert(0, _p)

import concourse.bacc as bacc
import concourse.tile as tile
from concourse import mybir
from concourse.bass_utils import run_bass_kernel_spmd

# Pin every activation to the one table set that contains all functions this
# kernel uses (Square/Ln/Exp/Copy/Abs/Identity). Without this, Bacc's
# first-match set selection alternates between sets (Ln lives outside the
# default exp set) and reloads the ACT tables ~1.3us per switch every tile.
_ACT_SET = "natural_log_exp_and_others"


def _pin_act_tables():
    import functools
    import concourse.hw_specs as hw_specs
    if getattr(hw_specs.get_activation_tables, "_pinned", False):
        return
    orig = hw_specs.get_activation_tables

    @functools.cache
    def pinned(arch):
        tabs = orig(arch)
        return {k: (v if k == _ACT_SET else set()) for k, v in tabs.items()}

    pinned._pinned = True
    hw_specs.get_activation_tables = pinned
    bacc.get_activation_tables = pinned


_pin_act_tables()

F32 = mybir.dt.float32
AF = mybir.ActivationFunctionType

# Problem constants (hardcoded per spec).
B = 64            # batch
K = 4             # n_cls
C = 256           # feature dim
NB = 8192         # bank entries per class (global)
NCORES = 8
NSH = NB // NCORES          # 1024 bank entries per class per core
ROWS = K * NSH              # 4096 rows of [4*256] per core
EPS_NLL = 1e-7
EPS_COH = 1e-8

# DMA tile layout: 512-entry transfers in steady state, the last class
# split 512/256/128/128 so the drain chain operates on few entries.
DMA_SIZES = [512, 512, 512, 512, 512, 512, 512, 256, 128, 128]
CHUNK_CLASS = [0, 0, 1, 1, 2, 2, 3, 3, 3, 3]   # class of each DMA tile/chunk
NCHUNK = len(DMA_SIZES)


def _build():
    nc = bacc.Bacc("TRN2", target_bir_lowering=False, debug=False,
                   enable_asserts=False, num_devices=NCORES)

    bank = nc.dram_tensor("bank", [ROWS, 1024], F32, kind="ExternalInput")
    indiv = nc.dram_tensor("indiv", [B, 1024], F32, kind="ExternalInput")
    gp = nc.dram_tensor("gp", [B, 512], F32, kind="ExternalInput")
    haz = nc.dram_tensor("haz", [B, K], F32, kind="ExternalInput")
    spad = nc.dram_tensor("spad", [B, K + 1], F32, kind="ExternalInput")
    ohy = nc.dram_tensor("ohy", [B, K + 1], F32, kind="ExternalInput")
    ohy1 = nc.dram_tensor("ohy1", [B, K + 1], F32, kind="ExternalInput")
    ohc = nc.dram_tensor("ohc", [B, NCHUNK], F32, kind="ExternalInput")
    cfs = nc.dram_tensor("cfs", [B, 2], F32, kind="ExternalInput")

    out_d = nc.dram_tensor("out_f", [B, 4], F32, kind="ExternalOutput")

    import ml_dtypes
    ident_bf_d = nc.inline_tensor(np.eye(128, dtype=ml_dtypes.bfloat16), "ident_bf")

    v = nc.vector
    a = nc.scalar

    with tile.TileContext(nc) as tc:
        from contextlib import ExitStack
        with ExitStack() as ctx:
            const = ctx.enter_context(tc.tile_pool(name="const", bufs=1))
            small = ctx.enter_context(tc.tile_pool(name="small", bufs=1))
            tpool = ctx.enter_context(tc.tile_pool(name="T", bufs=NCHUNK))
            spool = ctx.enter_context(tc.tile_pool(name="S", bufs=6))
            stpool = ctx.enter_context(tc.tile_pool(name="STsb", bufs=6))
            epool = ctx.enter_context(tc.tile_pool(name="esb", bufs=6))
            sqpool = ctx.enter_context(tc.tile_pool(name="sq", bufs=6))
            ps_st = ctx.enter_context(tc.tile_pool(name="ps_st", bufs=2, space="PSUM"))
            ps_p = ctx.enter_context(tc.tile_pool(name="ps_p", bufs=3, space="PSUM"))
            ps_one = ctx.enter_context(tc.tile_pool(name="ps_one", bufs=1, space="PSUM"))

            BF16 = mybir.dt.bfloat16
            # Small inputs go through the SAME SWDGE ring as the tile stream:
            # HWDGE transfers starve behind a saturated SWDGE stream (the 16
            # SDMA engines round-robin, and the stream has a deep backlog), so
            # the two loop-critical inputs are queued BEFORE the stream and
            # the scalar-term inputs are interleaved between tile issues.
            ident_bf = const.tile([128, 128], BF16)
            nc.gpsimd.dma_start(out=ident_bf[:], in_=ident_bf_d[:])
            # indiv/gp are cast to bf16 during DMA: halves the prologue's
            # big elementwise ops on DVE (anchor/intra numerics tolerate it)
            ind_sb = small.tile([B, 1024], BF16)
            nc.gpsimd.dma_start(out=ind_sb[:], in_=indiv[:])

            sm_shapes = {"haz": K, "spad": K + 1, "ohy": K + 1, "ohy1": K + 1,
                         "ohc": NCHUNK, "cfs": 2}
            sm_tiles = {n: small.tile([B, w], F32, name=f"sm_{n}")
                        for n, w in sm_shapes.items()}
            sm_tiles["gp"] = small.tile([B, 512], BF16, name="sm_gp")
            sm_src = {"haz": haz, "spad": spad, "ohy": ohy, "ohy1": ohy1,
                      "ohc": ohc, "cfs": cfs, "gp": gp}
            sm_sched = [["haz", "spad", "gp"], ["ohy", "ohy1"],
                        ["ohc", "cfs"]] + [[]] * (NCHUNK - 3)

            # ---------- hoisted DMA-issue block ----------
            # All DMA issues are emitted first in program order: with one
            # buffer per transfer none carries a buffer-recycle wait, so the
            # static schedule keeps the GPSIMD queue pure-DMA and the
            # stream self-paces at HBM line rate no matter what the
            # compute engines are doing.
            T_tiles = []
            row0 = 0
            for t, sz in enumerate(DMA_SIZES):
                epg = sz // 128
                T_sb = tpool.tile([128, 4096], BF16, name=f"T{t}", tag="T")
                src = bank[row0:row0 + sz, :].rearrange(
                    "(p e) x -> p e x", e=epg)
                row0 += sz
                # SWDGE cast-DMA: f32 HBM -> bf16 SBUF at line rate
                nc.gpsimd.dma_start(
                    out=T_sb[:, :epg * 1024].rearrange("p (e x) -> p e x",
                                                       e=epg),
                    in_=src)
                for name in sm_sched[t]:
                    nc.gpsimd.dma_start(out=sm_tiles[name][:],
                                        in_=sm_src[name][:])
                T_tiles.append(T_sb)

            # ---------- anchors: A = l2norm(mean_j indiv[b,j,:]) ----------
            # (must precede the loop: at_sb feeds every tile's matmul)
            iv = ind_sb.rearrange("p (j c) -> p j c", j=4)
            asum = small.tile([B, C], BF16)
            atmp = small.tile([B, C], BF16)
            v.tensor_add(asum[:], iv[:, 0, :], iv[:, 1, :])
            v.tensor_add(atmp[:], iv[:, 2, :], iv[:, 3, :])
            v.tensor_add(asum[:], asum[:], atmp[:])
            sqa = small.tile([B, C], F32)
            ssa = small.tile([B, 1], F32)
            a.activation(sqa[:], asum[:], AF.Square, accum_out=ssa[:])
            lna = small.tile([B, 1], F32)
            a.activation(lna[:], ssa[:], AF.Ln)
            rsa = small.tile([B, 1], F32)
            a.activation(rsa[:], lna[:], AF.Exp, scale=-0.5)
            v.tensor_scalar_mul(asum[:], asum[:], rsa[:])
            at_ps = ps_one.tile([128, 2, B], BF16, tag="at")
            for h in range(2):
                nc.tensor.transpose(at_ps[:, h, :],
                                    asum[:, h * 128:(h + 1) * 128],
                                    ident_bf[0:B, 0:B])
            at_sb = const.tile([128, 2, B], BF16)
            a.copy(at_sb[:], at_ps[:])

            # ---------- main loop over bank tiles ----------
            # 8 DMA transfers of 512 entries each (fewest ramps); the LAST
            # transfer's compute is chunked 256/128/128 along the e axis so
            # the serial drain chains after the HBM stream ends are short.
            # (Entries within a DMA tile all belong to one class, and the
            # e-axis chunking just partitions the entry set.)
            # et_all[:, t] collects each chunk's exp-sum straight from the
            # exp's accum_out — no per-chunk DVE accumulation op, so the
            # in-order DVE queue has no end-of-chain blocker and tile t+1's
            # adds run while tile t finishes on ACT/PE.
            et_all = small.tile([B, NCHUNK], F32)
            F = small.tile([B, 4], F32)
            v.memset(F[:], 0.0)
            cum = 0
            for t, sz in enumerate(DMA_SIZES):
                ne = sz // 128
                nt = sz
                cum += sz
                Tc = T_tiles[t][:, :ne * 1024].rearrange(
                    "p (e j c) -> p e j c", e=ne, j=4)
                # Pin tile t's compute to its real data-arrival time: the
                # scheduler's DMA cost model is optimistic, and the
                # resulting too-eager static order makes the in-order
                # engines lag the stream at the drain.
                tc.tile_set_cur_wait(0.010 + 0.0056 * (cum / 512))
                S_sb = spool.tile([128, 1024], BF16)
                Sv = S_sb[:, :ne * C].rearrange("p (e c) -> p e c", e=ne)
                tmp = spool.tile([128, 1024], BF16, tag="tmp")
                tv = tmp[:, :ne * C].rearrange("p (e c) -> p e c", e=ne)
                v.tensor_add(Sv[:], Tc[:, :, 0, :], Tc[:, :, 1, :])
                v.tensor_add(tv[:], Tc[:, :, 2, :], Tc[:, :, 3, :])
                v.tensor_add(Sv[:], Sv[:], tv[:])
                # sum-of-squares on DVE (Q7 compute interferes with the
                # SWDGE stream, so GPSIMD stays pure-DMA)
                sq = sqpool.tile([128, 1024], BF16)
                sqv = sq[:, :ne * C].rearrange("p (e c) -> p e c", e=ne)
                v.tensor_mul(sqv[:], Sv[:], Sv[:])
                ssum4 = spool.tile([128, 4], F32, tag="ssum")
                v.reduce_sum(ssum4[:, :ne], sqv[:], axis=mybir.AxisListType.X)
                rh4 = spool.tile([128, 4], F32, tag="rh4")
                a.activation(rh4[:, :ne], ssum4[:, :ne], AF.Ln)
                a.activation(rh4[:, :ne], rh4[:, :ne], AF.Exp, scale=-0.5)
                # normalize S rows in place (per-entry 1/||S||); split the
                # per-e scalings across DVE and ACT to balance load
                for e in range(ne):
                    if e % 2 == 0:
                        v.tensor_scalar_mul(Sv[:, e, :], Sv[:, e, :],
                                            rh4[:, e:e + 1])
                    else:
                        a.activation(Sv[:, e, :], Sv[:, e, :], AF.Copy,
                                     scale=rh4[:, e:e + 1])
                # transpose e-groups into [c, n] chunks (h = c-half)
                st_ps = [ps_st.tile([128, 512], BF16, name=f"stps{h}",
                                    tag=f"stps{h}") for h in range(2)]
                for e in range(ne):
                    for h in range(2):
                        nc.tensor.transpose(
                            st_ps[h][:, e * 128:(e + 1) * 128],
                            S_sb[:, e * C + h * 128: e * C + (h + 1) * 128],
                            ident_bf[:])
                p_ps = ps_p.tile([B, 512], F32)
                for h in range(2):
                    st_sb = stpool.tile([128, 512], BF16)
                    a.copy(st_sb[:, :nt], st_ps[h][:, :nt])
                    nc.tensor.matmul(p_ps[:, :nt], at_sb[:, h, :],
                                     st_sb[:, :nt],
                                     start=(h == 0), stop=(h == 1))
                e_sb = epool.tile([B, 512], F32)
                # exp(sims/tau); the row-sum lands directly in et_all[:, t]
                a.activation(e_sb[:, :nt], p_ps[:, :nt], AF.Exp, scale=0.5,
                             accum_out=et_all[:, t:t + 1])
                if t == 4:
                    contrib = _emit_scalar_terms(nc, tc, small, ind_sb,
                                                 sm_tiles)
            tc.cur_wait_ts = None

            # ---------- epilogue: per-b partial columns, host finishes ----
            ohc_sb = sm_tiles["ohc"]
            t4b = small.tile([B, NCHUNK], F32)
            v.tensor_mul(t4b[:], et_all[:], ohc_sb[:])
            v.reduce_sum(F[:, 0:1], t4b[:], axis=mybir.AxisListType.X)
            v.reduce_sum(F[:, 1:2], et_all[:], axis=mybir.AxisListType.X)
            v.tensor_copy(F[:, 2:3], contrib[:])
            nc.sync.dma_start(out=out_d[:], in_=F[:])

    nc.compile()
    return nc


def _emit_scalar_terms(nc, tc, small, ind_sb, sm):
    """NLL + intra terms (tiny [B,*] math).  Emitted mid-loop with sim-time
    pins at each chain's real input-arrival time, so the scheduler slots the
    long serial chains into true engine-idle windows of the HBM stream."""
    v = nc.vector
    a = nc.scalar
    contrib = _emit_intra(nc, tc, small, ind_sb, sm)

    # ---------- NLL (per-b, b on partitions) ----------
    tc.tile_set_cur_wait(0.027)
    haz_sb = sm["haz"]
    spad_sb = sm["spad"]
    ohy_sb = sm["ohy"]
    ohy1_sb = sm["ohy1"]
    cfs_sb = sm["cfs"]

    t5 = small.tile([B, K + 1], F32)
    t4 = small.tile([B, K], F32)
    sy = small.tile([B, 1], F32)
    hy = small.tile([B, 1], F32)
    sy1 = small.tile([B, 1], F32)
    v.tensor_mul(t5[:], spad_sb[:], ohy_sb[:])
    v.reduce_sum(sy[:], t5[:], axis=mybir.AxisListType.X)
    v.tensor_mul(t4[:], haz_sb[:], ohy_sb[:, 0:K])
    v.reduce_sum(hy[:], t4[:], axis=mybir.AxisListType.X)
    v.tensor_mul(t5[:], spad_sb[:], ohy1_sb[:])
    v.reduce_sum(sy1[:], t5[:], axis=mybir.AxisListType.X)
    for x in (sy, hy, sy1):
        v.tensor_scalar_max(x[:], x[:], EPS_NLL)
    lsy = small.tile([B, 1], F32)
    lhy = small.tile([B, 1], F32)
    lsy1 = small.tile([B, 1], F32)
    a.activation(lsy[:], sy[:], AF.Ln)
    a.activation(lhy[:], hy[:], AF.Ln)
    a.activation(lsy1[:], sy1[:], AF.Ln)
    tu = small.tile([B, 1], F32)
    tcen = small.tile([B, 1], F32)
    negl = small.tile([B, 1], F32)
    v.tensor_add(tu[:], lsy[:], lhy[:])
    v.tensor_mul(tu[:], tu[:], cfs_sb[:, 1:2])      # *(1-cf)
    v.tensor_mul(tcen[:], lsy1[:], cfs_sb[:, 0:1])  # *cf
    v.tensor_add(negl[:], tu[:], tcen[:])           # = -neg_l per b

    # combine: contrib_b = -negl/B + isum/(8B) + 1/B
    isum = contrib.pop("isum")
    c1 = small.tile([B, 1], F32)
    c2 = small.tile([B, 1], F32)
    cb = small.tile([B, 1], F32)
    v.tensor_scalar_mul(c1[:], negl[:], -1.0 / B)
    v.tensor_scalar_mul(c2[:], isum[:], 1.0 / (8 * B))
    v.tensor_add(cb[:], c1[:], c2[:])
    v.tensor_scalar_add(cb[:], cb[:], 1.0 / B)
    return cb


def _emit_intra(nc, tc, small, ind_sb, sm):
    v = nc.vector
    a = nc.scalar
    # ---------- intra cohort term (bf16 inputs) ----------
    tc.tile_set_cur_wait(0.019)
    BF16 = mybir.dt.bfloat16
    gp_sb = sm["gp"]
    sqi = small.tile([B, 1024], BF16)
    v.tensor_mul(sqi[:], ind_sb[:], ind_sb[:])
    ssqi = small.tile([B, 4], F32)
    v.reduce_sum(ssqi[:], sqi.rearrange("p (j c) -> p j c", j=4),
                 axis=mybir.AxisListType.X)
    rsi = small.tile([B, 4], F32)
    a.activation(rsi[:], ssqi[:], AF.Ln)
    a.activation(rsi[:], rsi[:], AF.Exp, scale=-0.5)
    sqg = small.tile([B, 512], BF16)
    v.tensor_mul(sqg[:], gp_sb[:], gp_sb[:])
    ssqg = small.tile([B, 2], F32)
    v.reduce_sum(ssqg[:], sqg.rearrange("p (t c) -> p t c", t=2),
                 axis=mybir.AxisListType.X)
    rsg = small.tile([B, 2], F32)
    a.activation(rsg[:], ssqg[:], AF.Ln)
    a.activation(rsg[:], rsg[:], AF.Exp, scale=-0.5)
    # normalize rows in place (anchor sums already consumed ind_sb)
    for p in range(4):
        v.tensor_scalar_mul(ind_sb[:, p * C:(p + 1) * C],
                            ind_sb[:, p * C:(p + 1) * C], rsi[:, p:p + 1])
    for t in range(2):
        v.tensor_scalar_mul(gp_sb[:, t * C:(t + 1) * C],
                            gp_sb[:, t * C:(t + 1) * C], rsg[:, t:t + 1])
    D = small.tile([B, 8], F32)
    prod = small.tile([B, C], BF16)
    for p in range(4):
        for t in range(2):
            col = p * 2 + t
            v.tensor_mul(prod[:], ind_sb[:, p * C:(p + 1) * C],
                         gp_sb[:, t * C:(t + 1) * C])
            v.reduce_sum(D[:, col:col + 1], prod[:],
                         axis=mybir.AxisListType.X)
    U = small.tile([B, 8], F32)
    a.activation(U[:], D[:], AF.Abs)
    # mask==1 entries (cols 0,1,4,7) use -sim instead of |sim|
    v.tensor_scalar_mul(U[:, 0:2], D[:, 0:2], -1.0)
    v.tensor_scalar_mul(U[:, 4:5], D[:, 4:5], -1.0)
    v.tensor_scalar_mul(U[:, 7:8], D[:, 7:8], -1.0)
    isum = small.tile([B, 1], F32)
    v.reduce_sum(isum[:], U[:], axis=mybir.AxisListType.X)
    return {"isum": isum}


_NC = None


def _get_nc():
    global _NC
    if _NC is None:
        _NC = _build()
    return _NC


def _make_in_maps(hazards, S, indiv, gene, path, cohort_bank, label, c):
    hazards = np.asarray(hazards, dtype=np.float32)
    S = np.asarray(S, dtype=np.float32)
    indiv = np.asarray(indiv, dtype=np.float32)
    gene = np.asarray(gene, dtype=np.float32)
    path = np.asarray(path, dtype=np.float32)
    cohort_bank = np.asarray(cohort_bank, dtype=np.float32)
    label = np.asarray(label)
    c = np.asarray(c)

    oh5 = np.zeros((B, K + 1), np.float32)
    oh5[np.arange(B), label] = 1.0
    oh5b = np.zeros((B, K + 1), np.float32)
    oh5b[np.arange(B), label + 1] = 1.0
    oh4 = oh5[:, :K]
    # per-chunk positive-class mask: chunk t holds entries of CHUNK_CLASS[t]
    ohc = np.ascontiguousarray(oh4[:, CHUNK_CLASS])
    spad = np.concatenate([np.ones((B, 1), np.float32), S], axis=1)
    cfs = np.stack([c.astype(np.float32), 1.0 - c.astype(np.float32)], axis=1)
    common = dict(
        indiv=np.ascontiguousarray(indiv.reshape(B, -1)),
        gp=np.ascontiguousarray(
            np.concatenate([gene.reshape(B, -1), path.reshape(B, -1)], axis=1)),
        haz=np.ascontiguousarray(hazards),
        spad=np.ascontiguousarray(spad),
        ohy=oh5, ohy1=oh5b, ohc=ohc, cfs=np.ascontiguousarray(cfs),
    )
    bankf = cohort_bank.reshape(K, NB, 1024)
    in_maps = []
    for i in range(NCORES):
        shard = np.ascontiguousarray(
            bankf[:, i * NSH:(i + 1) * NSH, :]).reshape(ROWS, 1024)
        in_maps.append({**common, "bank": shard})
    return in_maps


_LAST_RESULTS = None  # stashed for test.py introspection


def kernel(hazards, S, indiv, gene, path, cohort_bank, label, c):
    global _LAST_RESULTS
    nc = _get_nc()
    in_maps = _make_in_maps(hazards, S, indiv, gene, path, cohort_bank, label, c)
    trace = bool(int(os.environ.get("TRNK_TRACE", "0")))
    res = run_bass_kernel_spmd(nc, in_maps, core_ids=list(range(NCORES)),
                               trace=trace)
    _LAST_RESULTS = res
    outs = np.stack([r["out_f"] for r in res.results])  # [8, B, 4]
    ep_raw = float(outs[:, :, 0].sum())
    rsum_raw = float(outs[:, :, 1].sum())
    ep = ep_raw / (B * NB)
    en = (rsum_raw - ep_raw) / (B * (K - 1) * NB)
    other = float(outs[:, :, 2].sum(axis=1).mean())
    loss = other - math.log((ep + EPS_COH) / (ep + en + EPS_COH))
    return np.float32(loss)


# revision 38
# speedup vs baseline: 1.0583x; 1.0266x over previous
"""Trainium2 Bass kernel for nn_Loss_factory_12429635355015.

Loss = NLLSurv + CohortLoss(intra + inter) over a [4, 8192, 4, 256] cohort bank.

Strategy (memory-bound, 8 NeuronCores):
  - Shard cohort_bank along the N (bank-entry) axis: each core streams its
    16 MiB shard once at HBM line rate via SWDGE cast-DMAs (f32 -> bf16).
  - Per tile (n entries, 4 per partition): DVE does the component-sum and
    sum-of-squares, ACT does rsqrt + exp(+accum), PE transposes S and runs
    the anchor matmul.  Per-tile engine busy is kept below the ~5.75us/tile
    DMA floor so the HBM stream is the only bottleneck.
  - The scalar NLL + intra terms are emitted AFTER the main loop so their
    long serial dependency chains get LOW scheduler priority and fill idle
    engine slots instead of gating the DVE progress counters that recycle
    DMA buffers (deep tile pools absorb the remaining jitter).
  - Each core outputs [ep_partial, en_partial, nll+intra]; the host sums the
    two scalars across cores (the 'all-reduce two scalars' step) and applies
    the final -log((ep+eps)/(ep+en+eps)).
"""

import math
import os
import sys

import numpy as np

for _p in ("/opt/trn_rl_repo",):
    if _p not in sys.path and os.path.isdir(_p):
        sys.path.insert(0, _p)

import concourse.bacc as bacc
import concourse.tile as tile
from concourse import mybir
from concourse.bass_utils import run_bass_kernel_spmd

# Pin every activation to the one table set that contains all functions this
# kernel uses (Square/Ln/Exp/Copy/Abs/Identity). Without this, Bacc's
# first-match set selection alternates between sets (Ln lives outside the
# default exp set) and reloads the ACT tables ~1.3us per switch every tile.
_ACT_SET = "natural_log_exp_and_others"


def _pin_act_tables():
    import functools
    import concourse.hw_specs as hw_specs
    if getattr(hw_specs.get_activation_tables, "_pinned", False):
        return
    orig = hw_specs.get_activation_tables

    @functools.cache
    def pinned(arch):
        tabs = orig(arch)
        return {k: (v if k == _ACT_SET else set()) for k, v in tabs.items()}

    pinned._pinned = True
    hw_specs.get_activation_tables = pinned
    bacc.get_activation_tables = pinned


_pin_act_tables()

F32 = mybir.dt.float32
AF = mybir.ActivationFunctionType

# Problem constants (hardcoded per spec).
B = 64            # batch
K = 4             # n_cls
C = 256           # feature dim
NB = 8192         # bank entries per class (global)
NCORES = 8
NSH = NB // NCORES          # 1024 bank entries per class per core
ROWS = K * NSH              # 4096 rows of [4*256] per core
EPS_NLL = 1e-7
EPS_COH = 1e-8

# DMA tile layout: 512-entry transfers in steady state, the last class
# split 512/256/128/128 so the drain chain operates on few entries.
DMA_SIZES = [512, 512, 512, 512, 512, 512, 512, 256, 128, 128]
CHUNK_CLASS = [0, 0, 1, 1, 2, 2, 3, 3, 3, 3]   # class of each DMA tile/chunk
NCHUNK = len(DMA_SIZES)


def _build():
    nc = bacc.Bacc("TRN2", target_bir_lowering=False, debug=False,
                   enable_asserts=False, num_devices=NCORES)

    bank = nc.dram_tensor("bank", [ROWS, 1024], F32, kind="ExternalInput")
    indiv = nc.dram_tensor("indiv", [B, 1024], F32, kind="ExternalInput")
    gp = nc.dram_tensor("gp", [B, 512], F32, kind="ExternalInput")
    haz = nc.dram_tensor("haz", [B, K], F32, kind="ExternalInput")
    spad = nc.dram_tensor("spad", [B, K + 1], F32, kind="ExternalInput")
    ohy = nc.dram_tensor("ohy", [B, K + 1], F32, kind="ExternalInput")
    ohy1 = nc.dram_tensor("ohy1", [B, K + 1], F32, kind="ExternalInput")
    ohc = nc.dram_tensor("ohc", [B, NCHUNK], F32, kind="ExternalInput")
    cfs = nc.dram_tensor("cfs", [B, 2], F32, kind="ExternalInput")

    out_d = nc.dram_tensor("out_f", [B, 4], F32, kind="ExternalOutput")

    import ml_dtypes
    ident_bf_d = nc.inline_tensor(np.eye(128, dtype=ml_dtypes.bfloat16), "ident_bf")

    v = nc.vector
    a = nc.scalar

    with tile.TileContext(nc) as tc:
        from contextlib import ExitStack
        with ExitStack() as ctx:
            const = ctx.enter_context(tc.tile_pool(name="const", bufs=1))
            small = ctx.enter_context(tc.tile_pool(name="small", bufs=1))
            tpool = ctx.enter_context(tc.tile_pool(name="T", bufs=NCHUNK))
            spool = ctx.enter_context(tc.tile_pool(name="S", bufs=6))
            stpool = ctx.enter_context(tc.tile_pool(name="STsb", bufs=6))
            epool = ctx.enter_context(tc.tile_pool(name="esb", bufs=6))
            sqpool = ctx.enter_context(tc.tile_pool(name="sq", bufs=6))
            ps_st = ctx.enter_context(tc.tile_pool(name="ps_st", bufs=2, space="PSUM"))
            ps_p = ctx.enter_context(tc.tile_pool(name="ps_p", bufs=3, space="PSUM"))
            ps_one = ctx.enter_context(tc.tile_pool(name="ps_one", bufs=1, space="PSUM"))

            BF16 = mybir.dt.bfloat16
            # Small inputs go through the SAME SWDGE ring as the tile stream:
            # HWDGE transfers starve behind a saturated SWDGE stream (the 16
            # SDMA engines round-robin, and the stream has a deep backlog), so
            # the two loop-critical inputs are queued BEFORE the stream and
            # the scalar-term inputs are interleaved between tile issues.
            # ident_bf rides the idle HWDGE path — it completes before the
            # SWDGE stream saturates the SDMA engines
            ident_bf = const.tile([128, 128], BF16)
            nc.sync.dma_start(out=ident_bf[:], in_=ident_bf_d[:])
            # indiv/gp are cast to bf16 during DMA: halves the prologue's
            # big elementwise ops on DVE (anchor/intra numerics tolerate it)
            ind_sb = small.tile([B, 1024], BF16)
            nc.gpsimd.dma_start(out=ind_sb[:], in_=indiv[:])

            sm_shapes = {"haz": K, "spad": K + 1, "ohy": K + 1, "ohy1": K + 1,
                         "ohc": NCHUNK, "cfs": 2}
            sm_tiles = {n: small.tile([B, w], F32, name=f"sm_{n}")
                        for n, w in sm_shapes.items()}
            sm_tiles["gp"] = small.tile([B, 512], BF16, name="sm_gp")
            sm_src = {"haz": haz, "spad": spad, "ohy": ohy, "ohy1": ohy1,
                      "ohc": ohc, "cfs": cfs, "gp": gp}
            sm_sched = [["haz", "spad", "gp"], ["ohy", "ohy1"],
                        ["ohc", "cfs"]] + [[]] * (NCHUNK - 3)

            # ---------- hoisted DMA-issue block ----------
            # All DMA issues are emitted first in program order: with one
            # buffer per transfer none carries a buffer-recycle wait, so the
            # static schedule keeps the GPSIMD queue pure-DMA and the
            # stream self-paces at HBM line rate no matter what the
            # compute engines are doing.
            T_tiles = []
            row0 = 0
            for t, sz in enumerate(DMA_SIZES):
                epg = sz // 128
                T_sb = tpool.tile([128, 4096], BF16, name=f"T{t}", tag="T")
                src = bank[row0:row0 + sz, :].rearrange(
                    "(p e) x -> p e x", e=epg)
                row0 += sz
                # SWDGE cast-DMA: f32 HBM -> bf16 SBUF at line rate
                nc.gpsimd.dma_start(
                    out=T_sb[:, :epg * 1024].rearrange("p (e x) -> p e x",
                                                       e=epg),
                    in_=src)
                for name in sm_sched[t]:
                    nc.gpsimd.dma_start(out=sm_tiles[name][:],
                                        in_=sm_src[name][:])
                T_tiles.append(T_sb)

            # ---------- anchors: A = l2norm(mean_j indiv[b,j,:]) ----------
            # (must precede the loop: at_sb feeds every tile's matmul)
            iv = ind_sb.rearrange("p (j c) -> p j c", j=4)
            asum = small.tile([B, C], BF16)
            atmp = small.tile([B, C], BF16)
            v.tensor_add(asum[:], iv[:, 0, :], iv[:, 1, :])
            v.tensor_add(atmp[:], iv[:, 2, :], iv[:, 3, :])
            v.tensor_add(asum[:], asum[:], atmp[:])
            sqa = small.tile([B, C], F32)
            ssa = small.tile([B, 1], F32)
            a.activation(sqa[:], asum[:], AF.Square, accum_out=ssa[:])
            lna = small.tile([B, 1], F32)
            a.activation(lna[:], ssa[:], AF.Ln)
            rsa = small.tile([B, 1], F32)
            a.activation(rsa[:], lna[:], AF.Exp, scale=-0.5)
            v.tensor_scalar_mul(asum[:], asum[:], rsa[:])
            at_ps = ps_one.tile([128, 2, B], BF16, tag="at")
            for h in range(2):
                nc.tensor.transpose(at_ps[:, h, :],
                                    asum[:, h * 128:(h + 1) * 128],
                                    ident_bf[0:B, 0:B])
            at_sb = const.tile([128, 2, B], BF16)
            a.copy(at_sb[:], at_ps[:])

            # ---------- main loop over bank tiles ----------
            # 8 DMA transfers of 512 entries each (fewest ramps); the LAST
            # transfer's compute is chunked 256/128/128 along the e axis so
            # the serial drain chains after the HBM stream ends are short.
            # (Entries within a DMA tile all belong to one class, and the
            # e-axis chunking just partitions the entry set.)
            # et_all[:, t] collects each chunk's exp-sum straight from the
            # exp's accum_out — no per-chunk DVE accumulation op, so the
            # in-order DVE queue has no end-of-chain blocker and tile t+1's
            # adds run while tile t finishes on ACT/PE.
            et_all = small.tile([B, NCHUNK], F32)
            F = small.tile([B, 4], F32)
            v.memset(F[:], 0.0)
            cum = 0
            for t, sz in enumerate(DMA_SIZES):
                ne = sz // 128
                nt = sz
                cum += sz
                Tc = T_tiles[t][:, :ne * 1024].rearrange(
                    "p (e j c) -> p e j c", e=ne, j=4)
                # Pin tile t's compute to its real data-arrival time: the
                # scheduler's DMA cost model is optimistic, and the
                # resulting too-eager static order makes the in-order
                # engines lag the stream at the drain.
                tc.tile_set_cur_wait(0.010 + 0.0056 * (cum / 512))
                S_sb = spool.tile([128, 1024], BF16)
                Sv = S_sb[:, :ne * C].rearrange("p (e c) -> p e c", e=ne)
                tmp = spool.tile([128, 1024], BF16, tag="tmp")
                tv = tmp[:, :ne * C].rearrange("p (e c) -> p e c", e=ne)
                v.tensor_add(Sv[:], Tc[:, :, 0, :], Tc[:, :, 1, :])
                v.tensor_add(tv[:], Tc[:, :, 2, :], Tc[:, :, 3, :])
                v.tensor_add(Sv[:], Sv[:], tv[:])
                # sum-of-squares on DVE (Q7 compute interferes with the
                # SWDGE stream, so GPSIMD stays pure-DMA)
                sq = sqpool.tile([128, 1024], BF16)
                sqv = sq[:, :ne * C].rearrange("p (e c) -> p e c", e=ne)
                v.tensor_mul(sqv[:], Sv[:], Sv[:])
                ssum4 = spool.tile([128, 4], F32, tag="ssum")
                v.reduce_sum(ssum4[:, :ne], sqv[:], axis=mybir.AxisListType.X)
                rh4 = spool.tile([128, 4], F32, tag="rh4")
                a.activation(rh4[:, :ne], ssum4[:, :ne], AF.Ln)
                a.activation(rh4[:, :ne], rh4[:, :ne], AF.Exp, scale=-0.5)
                # normalize S rows in place (per-entry 1/||S||); split the
                # per-e scalings across DVE and ACT to balance load
                for e in range(ne):
                    if e % 2 == 0:
                        v.tensor_scalar_mul(Sv[:, e, :], Sv[:, e, :],
                                            rh4[:, e:e + 1])
                    else:
                        a.activation(Sv[:, e, :], Sv[:, e, :], AF.Copy,
                                     scale=rh4[:, e:e + 1])
                # transpose e-groups into [c, n] chunks (h = c-half)
                st_ps = [ps_st.tile([128, 512], BF16, name=f"stps{h}",
                                    tag=f"stps{h}") for h in range(2)]
                for e in range(ne):
                    for h in range(2):
                        nc.tensor.transpose(
                            st_ps[h][:, e * 128:(e + 1) * 128],
                            S_sb[:, e * C + h * 128: e * C + (h + 1) * 128],
                            ident_bf[:])
                p_ps = ps_p.tile([B, 512], F32)
                for h in range(2):
                    st_sb = stpool.tile([128, 512], BF16)
                    if ne == 4:
                        a.copy(st_sb[:, :nt], st_ps[h][:, :nt])
                    else:
                        # drain chunks: keep the serial ACT chain short
                        v.tensor_copy(st_sb[:, :nt], st_ps[h][:, :nt])
                    nc.tensor.matmul(p_ps[:, :nt], at_sb[:, h, :],
                                     st_sb[:, :nt],
                                     start=(h == 0), stop=(h == 1))
                e_sb = epool.tile([B, 512], F32)
                # exp(sims/tau); the row-sum lands directly in et_all[:, t]
                a.activation(e_sb[:, :nt], p_ps[:, :nt], AF.Exp, scale=0.5,
                             accum_out=et_all[:, t:t + 1])
                if t == 4:
                    contrib = _emit_scalar_terms(nc, tc, small, ind_sb,
                                                 sm_tiles)
            tc.cur_wait_ts = None

            # ---------- epilogue: per-b partial columns, host finishes ----
            ohc_sb = sm_tiles["ohc"]
            t4b = small.tile([B, NCHUNK], F32)
            v.tensor_mul(t4b[:], et_all[:], ohc_sb[:])
            v.reduce_sum(F[:, 0:1], t4b[:], axis=mybir.AxisListType.X)
            v.reduce_sum(F[:, 1:2], et_all[:], axis=mybir.AxisListType.X)
            v.tensor_copy(F[:, 2:3], contrib[:])
            nc.sync.dma_start(out=out_d[:], in_=F[:])

    nc.compile()
    return nc


def _emit_scalar_terms(nc, tc, small, ind_sb, sm):
    """NLL + intra terms (tiny [B,*] math).  Emitted mid-loop with sim-time
    pins at each chain's real input-arrival time, so the scheduler slots the
    long serial chains into true engine-idle windows of the HBM stream."""
    v = nc.vector
    a = nc.scalar
    contrib = _emit_intra(nc, tc, small, ind_sb, sm)

    # ---------- NLL (per-b, b on partitions) ----------
    tc.tile_set_cur_wait(0.027)
    haz_sb = sm["haz"]
    spad_sb = sm["spad"]
    ohy_sb = sm["ohy"]
    ohy1_sb = sm["ohy1"]
    cfs_sb = sm["cfs"]

    t5 = small.tile([B, K + 1], F32)
    t4 = small.tile([B, K], F32)
    sy = small.tile([B, 1], F32)
    hy = small.tile([B, 1], F32)
    sy1 = small.tile([B, 1], F32)
    v.tensor_mul(t5[:], spad_sb[:], ohy_sb[:])
    v.reduce_sum(sy[:], t5[:], axis=mybir.AxisListType.X)
    v.tensor_mul(t4[:], haz_sb[:], ohy_sb[:, 0:K])
    v.reduce_sum(hy[:], t4[:], axis=mybir.AxisListType.X)
    v.tensor_mul(t5[:], spad_sb[:], ohy1_sb[:])
    v.reduce_sum(sy1[:], t5[:], axis=mybir.AxisListType.X)
    for x in (sy, hy, sy1):
        v.tensor_scalar_max(x[:], x[:], EPS_NLL)
    lsy = small.tile([B, 1], F32)
    lhy = small.tile([B, 1], F32)
    lsy1 = small.tile([B, 1], F32)
    a.activation(lsy[:], sy[:], AF.Ln)
    a.activation(lhy[:], hy[:], AF.Ln)
    a.activation(lsy1[:], sy1[:], AF.Ln)
    tu = small.tile([B, 1], F32)
    tcen = small.tile([B, 1], F32)
    negl = small.tile([B, 1], F32)
    v.tensor_add(tu[:], lsy[:], lhy[:])
    v.tensor_mul(tu[:], tu[:], cfs_sb[:, 1:2])      # *(1-cf)
    v.tensor_mul(tcen[:], lsy1[:], cfs_sb[:, 0:1])  # *cf
    v.tensor_add(negl[:], tu[:], tcen[:])           # = -neg_l per b

    # combine: contrib_b = -negl/B + isum/(8B) + 1/B
    isum = contrib.pop("isum")
    c1 = small.tile([B, 1], F32)
    c2 = small.tile([B, 1], F32)
    cb = small.tile([B, 1], F32)
    v.tensor_scalar_mul(c1[:], negl[:], -1.0 / B)
    v.tensor_scalar_mul(c2[:], isum[:], 1.0 / (8 * B))
    v.tensor_add(cb[:], c1[:], c2[:])
    v.tensor_scalar_add(cb[:], cb[:], 1.0 / B)
    return cb


def _emit_intra(nc, tc, small, ind_sb, sm):
    v = nc.vector
    a = nc.scalar
    # ---------- intra cohort term (bf16 inputs) ----------
    tc.tile_set_cur_wait(0.019)
    BF16 = mybir.dt.bfloat16
    gp_sb = sm["gp"]
    sqi = small.tile([B, 1024], BF16)
    v.tensor_mul(sqi[:], ind_sb[:], ind_sb[:])
    ssqi = small.tile([B, 4], F32)
    v.reduce_sum(ssqi[:], sqi.rearrange("p (j c) -> p j c", j=4),
                 axis=mybir.AxisListType.X)
    rsi = small.tile([B, 4], F32)
    a.activation(rsi[:], ssqi[:], AF.Ln)
    a.activation(rsi[:], rsi[:], AF.Exp, scale=-0.5)
    sqg = small.tile([B, 512], BF16)
    v.tensor_mul(sqg[:], gp_sb[:], gp_sb[:])
    ssqg = small.tile([B, 2], F32)
    v.reduce_sum(ssqg[:], sqg.rearrange("p (t c) -> p t c", t=2),
                 axis=mybir.AxisListType.X)
    rsg = small.tile([B, 2], F32)
    a.activation(rsg[:], ssqg[:], AF.Ln)
    a.activation(rsg[:], rsg[:], AF.Exp, scale=-0.5)
    # normalize rows in place (anchor sums already consumed ind_sb)
    for p in range(4):
        v.tensor_scalar_mul(ind_sb[:, p * C:(p + 1) * C],
                            ind_sb[:, p * C:(p + 1) * C], rsi[:, p:p + 1])
    for t in range(2):
        v.tensor_scalar_mul(gp_sb[:, t * C:(t + 1) * C],
                            gp_sb[:, t * C:(t + 1) * C], rsg[:, t:t + 1])
    D = small.tile([B, 8], F32)
    prod = small.tile([B, C], BF16)
    for p in range(4):
        for t in range(2):
            col = p * 2 + t
            v.tensor_mul(prod[:], ind_sb[:, p * C:(p + 1) * C],
                         gp_sb[:, t * C:(t + 1) * C])
            v.reduce_sum(D[:, col:col + 1], prod[:],
                         axis=mybir.AxisListType.X)
    U = small.tile([B, 8], F32)
    a.activation(U[:], D[:], AF.Abs)
    # mask==1 entries (cols 0,1,4,7) use -sim instead of |sim|
    v.tensor_scalar_mul(U[:, 0:2], D[:, 0:2], -1.0)
    v.tensor_scalar_mul(U[:, 4:5], D[:, 4:5], -1.0)
    v.tensor_scalar_mul(U[:, 7:8], D[:, 7:8], -1.0)
    isum = small.tile([B, 1], F32)
    v.reduce_sum(isum[:], U[:], axis=mybir.AxisListType.X)
    return {"isum": isum}


_NC = None


def _get_nc():
    global _NC
    if _NC is None:
        _NC = _build()
    return _NC


def _make_in_maps(hazards, S, indiv, gene, path, cohort_bank, label, c):
    hazards = np.asarray(hazards, dtype=np.float32)
    S = np.asarray(S, dtype=np.float32)
    indiv = np.asarray(indiv, dtype=np.float32)
    gene = np.asarray(gene, dtype=np.float32)
    path = np.asarray(path, dtype=np.float32)
    cohort_bank = np.asarray(cohort_bank, dtype=np.float32)
    label = np.asarray(label)
    c = np.asarray(c)

    oh5 = np.zeros((B, K + 1), np.float32)
    oh5[np.arange(B), label] = 1.0
    oh5b = np.zeros((B, K + 1), np.float32)
    oh5b[np.arange(B), label + 1] = 1.0
    oh4 = oh5[:, :K]
    # per-chunk positive-class mask: chunk t holds entries of CHUNK_CLASS[t]
    ohc = np.ascontiguousarray(oh4[:, CHUNK_CLASS])
    spad = np.concatenate([np.ones((B, 1), np.float32), S], axis=1)
    cfs = np.stack([c.astype(np.float32), 1.0 - c.astype(np.float32)], axis=1)
    common = dict(
        indiv=np.ascontiguousarray(indiv.reshape(B, -1)),
        gp=np.ascontiguousarray(
            np.concatenate([gene.reshape(B, -1), path.reshape(B, -1)], axis=1)),
        haz=np.ascontiguousarray(hazards),
        spad=np.ascontiguousarray(spad),
        ohy=oh5, ohy1=oh5b, ohc=ohc, cfs=np.ascontiguousarray(cfs),
    )
    bankf = cohort_bank.reshape(K, NB, 1024)
    in_maps = []
    for i in range(NCORES):
        shard = np.ascontiguousarray(
            bankf[:, i * NSH:(i + 1) * NSH, :]).reshape(ROWS, 1024)
        in_maps.append({**common, "bank": shard})
    return in_maps


_LAST_RESULTS = None  # stashed for test.py introspection


def kernel(hazards, S, indiv, gene, path, cohort_bank, label, c):
    global _LAST_RESULTS
    nc = _get_nc()
    in_maps = _make_in_maps(hazards, S, indiv, gene, path, cohort_bank, label, c)
    trace = bool(int(os.environ.get("TRNK_TRACE", "0")))
    res = run_bass_kernel_spmd(nc, in_maps, core_ids=list(range(NCORES)),
                               trace=trace)
    _LAST_RESULTS = res
    outs = np.stack([r["out_f"] for r in res.results])  # [8, B, 4]
    ep_raw = float(outs[:, :, 0].sum())
    rsum_raw = float(outs[:, :, 1].sum())
    ep = ep_raw / (B * NB)
    en = (rsum_raw - ep_raw) / (B * (K - 1) * NB)
    other = float(outs[:, :, 2].sum(axis=1).mean())
    loss = other - math.log((ep + EPS_COH) / (ep + en + EPS_COH))
    return np.float32(loss)
